# revision 4
# baseline (speedup 1.0000x reference)
"""Block-sparse (block-diagonal local) attention head for Trainium2, 8-way
data-parallel over the batch dimension (one batch element per NeuronCore).

Contract: kernel(**inputs) takes the FULL inputs from setup_inputs() and
returns the FULL output of reference(): out [8, 4096, 128] float32.

Per-core math (batch b):
  qT = (x_b @ Wq)^T, kT = (x_b @ Wk)^T, vT = (x_b @ Wv)^T   (Dh on partitions)
  per 128-token block j:
    v_j   = transpose(vT_j)                    (PE transpose, token-major)
    sT_j  = kT_j^T-contract: scoresT[k,q] = sum_d kT[d,k] qT[d,q]
    PT_j  = exp(sT_j / sqrt(Dh))               (no max-subtraction; logits are
                                                O(10) for this problem, exact
                                                in fp32 softmax algebra)
    o'_j  = PT_j^T @ [v_j | 1]                 (ones column gives the row sums)
    out_j = o'_j[:, :128] * (1 / o'_j[:, 128])

The host side pre-transposes x[b] to xT [D, S] so every device DMA is
contiguous, and shards batch b -> core b.
"""

import numpy as np
from contextlib import ExitStack

B, S, D, Dh, BLOCK = 8, 4096, 1024, 128, 128
KC = D // 128  # contraction chunks of 128
MT = 512       # token tile (moving free dim of projection matmuls)
NMT = S // MT
JT = MT // BLOCK
SCALE = float(1.0 / np.sqrt(np.float32(Dh)))

# matmul dtype mode for the projection matmuls: "f32" (exact, 4 cyc/row),
# "f32r" (fast fp32 mode, 1 cyc/row at free-dim >= 256), "bf16"
MM_MODE = "f32r"

_CACHE = {}


def _build(mode):
    import concourse.bass as bass
    import concourse.mybir as mybir
    import concourse.tile as tile
    from concourse import bacc
    from concourse.masks import make_identity

    f32 = mybir.dt.float32
    f32r = mybir.dt.float32r
    bf16 = mybir.dt.bfloat16
    ts = bass.ts

    # dtype of the x/W data path feeding the projection matmuls
    proj_dt = {"f32": f32, "f32r": f32r, "bf16": bf16}[mode]
    # dtype of SBUF staging tiles for everything downstream (attention)
    data_dt = bf16 if mode == "bf16" else f32

    nc = bacc.Bacc("TRN2", target_bir_lowering=False, debug=False)

    xT = nc.dram_tensor("xT", [D, S], proj_dt, kind="ExternalInput").ap()
    wq = nc.dram_tensor("wq", [D, Dh], proj_dt, kind="ExternalInput").ap()
    wk = nc.dram_tensor("wk", [D, Dh], proj_dt, kind="ExternalInput").ap()
    wv = nc.dram_tensor("wv", [D, Dh], proj_dt, kind="ExternalInput").ap()
    out = nc.dram_tensor("out", [S, Dh], f32, kind="ExternalOutput").ap()

    with tile.TileContext(nc) as tc, ExitStack() as ctx:
        wpool = ctx.enter_context(tc.tile_pool(name="w", bufs=1))
        cpool = ctx.enter_context(tc.tile_pool(name="const", bufs=1))
        xpool = ctx.enter_context(tc.tile_pool(name="x", bufs=2))
        spool = ctx.enter_context(tc.tile_pool(name="s", bufs=2))
        apool = ctx.enter_context(tc.tile_pool(name="a", bufs=3))
        ppool = ctx.enter_context(
            tc.tile_pool(name="proj_ps", bufs=3, space="PSUM")
        )
        qpool = ctx.enter_context(
            tc.tile_pool(name="attn_ps", bufs=5, space="PSUM")
        )

        # Identity for PE transposes.
        ident = cpool.tile([128, 128], data_dt, tag="ident")
        make_identity(nc, ident[:])

        # Weights, chunked along the contraction dim: w_t[:, k, :] is the
        # [128, Dh] stationary tile for contraction chunk k.
        w_ts = []
        for name, w in (("wq", wq), ("wk", wk), ("wv", wv)):
            w_t = wpool.tile([128, KC, Dh], proj_dt, tag=f"w_{name}")
            nc.sync.dma_start(w_t[:], w.rearrange("(k p) d -> p k d", p=128))
            w_ts.append(w_t)
        wq_t, wk_t, wv_t = w_ts

        for mt in range(NMT):
            m0 = mt * MT
            # x^T slice for this token tile: [128, KC, MT]
            xt = xpool.tile([128, KC, MT], proj_dt, tag="xt")
            nc.sync.dma_start(
                xt[:], xT.rearrange("(k p) s -> p k s", p=128)[:, :, m0 : m0 + MT]
            )

            # Projections (Dh on partitions): pT[d, m] = sum_k W[k, d] xT[k, m]
            pT_sbs = []
            for w_t, tag, copy_eng in (
                (wq_t, "qT", nc.vector),
                (wk_t, "kT", nc.scalar),
                (wv_t, "vT", nc.vector),
            ):
                pT_ps = ppool.tile([128, MT], f32, tag="proj")
                for k in range(KC):
                    nc.tensor.matmul(
                        pT_ps[:],
                        w_t[:, k, :],
                        xt[:, k, :],
                        start=(k == 0),
                        stop=(k == KC - 1),
                    )
                pT_sb = spool.tile([128, MT], data_dt, tag=tag)
                if copy_eng is nc.scalar:
                    nc.scalar.copy(pT_sb[:], pT_ps[:])
                else:
                    nc.vector.tensor_copy(pT_sb[:], pT_ps[:])
                pT_sbs.append(pT_sb)
            qT_sb, kT_sb, vT_sb = pT_sbs

            # Attention on the JT blocks of this token tile.
            for j in range(JT):
                blk = ts(j, BLOCK)
                # token-major v block via PE transpose
                v_ps = qpool.tile([128, BLOCK], f32, tag="attn")
                nc.tensor.transpose(v_ps[:], vT_sb[:, blk], ident[:])
                v_sb = apool.tile([128, BLOCK + 1], data_dt, tag="v")
                nc.vector.tensor_copy(v_sb[:, 0:BLOCK], v_ps[:])
                nc.vector.memset(v_sb[:, BLOCK : BLOCK + 1], 1.0)

                # scoresT[k, q] = sum_d kT[d, k] qT[d, q]
                sT_ps = qpool.tile([128, BLOCK], f32, tag="attn")
                nc.tensor.matmul(
                    sT_ps[:], kT_sb[:, blk], qT_sb[:, blk], start=True, stop=True
                )
                PT = apool.tile([128, BLOCK], data_dt, tag="PT")
                nc.scalar.activation(
                    PT[:], sT_ps[:], mybir.ActivationFunctionType.Exp, scale=SCALE
                )

                # o'[q, :Dh] = unnormalized attn output; o'[q, Dh] = row sum
                o_ps = qpool.tile([128, BLOCK + 1], f32, tag="attn")
                nc.tensor.matmul(o_ps[:], PT[:], v_sb[:], start=True, stop=True)

                r_sb = apool.tile([128, 1], f32, tag="r")
                nc.vector.reciprocal(r_sb[:], o_ps[:, BLOCK : BLOCK + 1])
                o_sb = apool.tile([128, BLOCK], f32, tag="o")
                nc.vector.tensor_scalar_mul(o_sb[:], o_ps[:, 0:BLOCK], r_sb[:])
                nc.sync.dma_start(out[m0 + j * BLOCK : m0 + (j + 1) * BLOCK, :], o_sb[:])

    nc.compile()
    return nc


def _get_nc():
    if MM_MODE not in _CACHE:
        _CACHE[MM_MODE] = _build(MM_MODE)
    return _CACHE[MM_MODE]


def kernel(x, Wq, Wk, Wv):
    from concourse.bass_utils import run_bass_kernel_spmd

    nc = _get_nc()
    if MM_MODE == "bf16":
        import ml_dtypes

        cast = lambda a: np.asarray(a, dtype=ml_dtypes.bfloat16)
    else:
        cast = lambda a: np.asarray(a, dtype=np.float32)

    wq_h, wk_h, wv_h = cast(Wq), cast(Wk), cast(Wv)
    in_maps = [
        {
            "xT": np.ascontiguousarray(cast(np.asarray(x[b]).T)),
            "wq": wq_h,
            "wk": wk_h,
            "wv": wv_h,
        }
        for b in range(B)
    ]
    res = run_bass_kernel_spmd(nc, in_maps, list(range(B))).results
    return np.stack([res[b]["out"] for b in range(B)], axis=0)


# revision 8
# speedup vs baseline: 1.0363x; 1.0363x over previous
"""Block-sparse (block-diagonal local) attention head for Trainium2, 8-way
data-parallel over the batch dimension (one batch element per NeuronCore).

Contract: kernel(**inputs) takes the FULL inputs from setup_inputs() and
returns the FULL output of reference(): out [8, 4096, 128] float32.

Per-core math (batch b):
  qT = (x_b @ Wq)^T, kT = (x_b @ Wk)^T, vT = (x_b @ Wv)^T   (Dh on partitions)
  per 128-token block j:
    v_j   = transpose(vT_j)                    (PE transpose, token-major)
    sT_j  = kT_j^T-contract: scoresT[k,q] = sum_d kT[d,k] qT[d,q]
    PT_j  = exp(sT_j / sqrt(Dh))               (no max-subtraction; logits are
                                                O(10) for this problem, exact
                                                in fp32 softmax algebra)
    o'_j  = PT_j^T @ [v_j | 1]                 (ones column gives the row sums)
    out_j = o'_j[:, :128] * (1 / o'_j[:, 128])

The host side pre-transposes x[b] to xT [D, S] so every device DMA is
contiguous, and shards batch b -> core b.
"""

import numpy as np
from contextlib import ExitStack

B, S, D, Dh, BLOCK = 8, 4096, 1024, 128, 128
KC = D // 128  # contraction chunks of 128
MT = 512       # token tile (moving free dim of projection matmuls)
NMT = S // MT
JT = MT // BLOCK
SCALE = float(1.0 / np.sqrt(np.float32(Dh)))

# matmul dtype mode for the projection matmuls: "f32" (exact, 4 cyc/row),
# "f32r" (fast fp32 mode, 1 cyc/row at free-dim >= 256), "bf16"
MM_MODE = "f32r"

_CACHE = {}


def _build(mode):
    import concourse.bass as bass
    import concourse.mybir as mybir
    import concourse.tile as tile
    from concourse import bacc

    f32 = mybir.dt.float32
    f32r = mybir.dt.float32r
    bf16 = mybir.dt.bfloat16
    ts = bass.ts

    # dtype of the x/W data path feeding the projection matmuls
    proj_dt = {"f32": f32, "f32r": f32r, "bf16": bf16}[mode]
    # dtype of SBUF staging tiles for the attention chain. f32r keeps the
    # attention matmuls single-pass on the PE (fp32 lowers to 2-pass).
    data_dt = bf16 if mode == "bf16" else f32r

    nc = bacc.Bacc("TRN2", target_bir_lowering=False, debug=False)

    xT = nc.dram_tensor("xT", [D, S], proj_dt, kind="ExternalInput").ap()
    ident_d = nc.dram_tensor("ident", [128, 128], data_dt, kind="ExternalInput").ap()
    wq = nc.dram_tensor("wq", [D, Dh], proj_dt, kind="ExternalInput").ap()
    wk = nc.dram_tensor("wk", [D, Dh], proj_dt, kind="ExternalInput").ap()
    wv = nc.dram_tensor("wv", [D, Dh], proj_dt, kind="ExternalInput").ap()
    out = nc.dram_tensor("out", [S, Dh], f32, kind="ExternalOutput").ap()

    with tile.TileContext(nc) as tc, ExitStack() as ctx:
        wpool = ctx.enter_context(tc.tile_pool(name="w", bufs=1))
        cpool = ctx.enter_context(tc.tile_pool(name="const", bufs=1))
        xpool = ctx.enter_context(tc.tile_pool(name="x", bufs=2))
        spool = ctx.enter_context(tc.tile_pool(name="s", bufs=2))
        apool = ctx.enter_context(tc.tile_pool(name="a", bufs=3))
        ppool = ctx.enter_context(
            tc.tile_pool(name="proj_ps", bufs=3, space="PSUM")
        )
        qpool = ctx.enter_context(
            tc.tile_pool(name="attn_ps", bufs=5, space="PSUM")
        )

        # Identity for PE transposes (from host: affine_select cannot
        # legally produce f32r-typed data for the transpose matmult).
        ident = cpool.tile([128, 128], data_dt, tag="ident")
        nc.sync.dma_start(ident[:], ident_d[:])

        # Weights, chunked along the contraction dim: w_t[:, k, :] is the
        # [128, Dh] stationary tile for contraction chunk k.
        w_ts = []
        for name, w in (("wq", wq), ("wk", wk), ("wv", wv)):
            w_t = wpool.tile([128, KC, Dh], proj_dt, tag=f"w_{name}")
            nc.sync.dma_start(w_t[:], w.rearrange("(k p) d -> p k d", p=128))
            w_ts.append(w_t)
        wq_t, wk_t, wv_t = w_ts

        for mt in range(NMT):
            m0 = mt * MT
            # x^T slice for this token tile: [128, KC, MT]
            xt = xpool.tile([128, KC, MT], proj_dt, tag="xt")
            xTv = xT.rearrange("(k p) s -> p k s", p=128)
            for k in range(KC):
                nc.sync.dma_start(xt[:, k, :], xTv[:, k, m0 : m0 + MT])

            # Projections (Dh on partitions): pT[d, m] = sum_k W[k, d] xT[k, m]
            pT_sbs = []
            for w_t, tag, copy_eng in (
                (wq_t, "qT", nc.vector),
                (wk_t, "kT", nc.scalar),
                (wv_t, "vT", nc.vector),
            ):
                pT_ps = ppool.tile([128, MT], f32, tag="proj")
                for k in range(KC):
                    nc.tensor.matmul(
                        pT_ps[:],
                        w_t[:, k, :],
                        xt[:, k, :],
                        start=(k == 0),
                        stop=(k == KC - 1),
                    )
                pT_sb = spool.tile([128, MT], data_dt, tag=tag)
                if copy_eng is nc.scalar:
                    nc.scalar.copy(pT_sb[:], pT_ps[:])
                else:
                    nc.vector.tensor_copy(pT_sb[:], pT_ps[:])
                pT_sbs.append(pT_sb)
            qT_sb, kT_sb, vT_sb = pT_sbs

            # Attention on the JT blocks of this token tile.
            for j in range(JT):
                blk = ts(j, BLOCK)
                # token-major v block via PE transpose
                v_ps = qpool.tile([128, BLOCK], data_dt, tag="attn")
                nc.tensor.transpose(v_ps[:], vT_sb[:, blk], ident[:])
                # f32r matmuls require an even moving free dim, so pad the
                # ones column to two (the second is ignored).
                v_sb = apool.tile([128, BLOCK + 2], data_dt, tag="v")
                nc.vector.tensor_copy(v_sb[:, 0:BLOCK], v_ps[:])
                ones_col = v_sb[:, BLOCK : BLOCK + 2]
                nc.vector.memset(
                    ones_col.bitcast(f32) if data_dt == f32r else ones_col, 1.0
                )

                # scoresT[k, q] = sum_d kT[d, k] qT[d, q]
                sT_ps = qpool.tile([128, BLOCK], f32, tag="attn")
                nc.tensor.matmul(
                    sT_ps[:], kT_sb[:, blk], qT_sb[:, blk], start=True, stop=True
                )
                PT = apool.tile([128, BLOCK], data_dt, tag="PT")
                nc.scalar.activation(
                    PT[:], sT_ps[:], mybir.ActivationFunctionType.Exp, scale=SCALE
                )

                # o'[q, :Dh] = unnormalized attn output; o'[q, Dh] = row sum
                o_ps = qpool.tile([128, BLOCK + 2], f32, tag="attn")
                nc.tensor.matmul(o_ps[:], PT[:], v_sb[:], start=True, stop=True)

                r_sb = apool.tile([128, 1], f32, tag="r")
                nc.vector.reciprocal(r_sb[:], o_ps[:, BLOCK : BLOCK + 1])
                o_sb = apool.tile([128, BLOCK], f32, tag="o")
                nc.vector.tensor_scalar_mul(o_sb[:], o_ps[:, 0:BLOCK], r_sb[:])
                nc.sync.dma_start(out[m0 + j * BLOCK : m0 + (j + 1) * BLOCK, :], o_sb[:])

    nc.compile()
    return nc


def _get_nc():
    if MM_MODE not in _CACHE:
        _CACHE[MM_MODE] = _build(MM_MODE)
    return _CACHE[MM_MODE]


def kernel(x, Wq, Wk, Wv):
    from concourse.bass_utils import run_bass_kernel_spmd

    nc = _get_nc()
    if MM_MODE == "bf16":
        import ml_dtypes

        cast = lambda a: np.asarray(a, dtype=ml_dtypes.bfloat16)
    else:
        cast = lambda a: np.asarray(a, dtype=np.float32)

    wq_h, wk_h, wv_h = cast(Wq), cast(Wk), cast(Wv)
    ident_h = cast(np.eye(128, dtype=np.float32))
    in_maps = [
        {
            "xT": np.ascontiguousarray(cast(np.asarray(x[b]).T)),
            "wq": wq_h,
            "wk": wk_h,
            "wv": wv_h,
            "ident": ident_h,
        }
        for b in range(B)
    ]
    res = run_bass_kernel_spmd(nc, in_maps, list(range(B))).results
    return np.stack([res[b]["out"] for b in range(B)], axis=0)


# revision 9
# speedup vs baseline: 1.1346x; 1.0948x over previous
"""Block-sparse (block-diagonal local) attention head for Trainium2, 8-way
data-parallel over the batch dimension (one batch element per NeuronCore).

Contract: kernel(**inputs) takes the FULL inputs from setup_inputs() and
returns the FULL output of reference(): out [8, 4096, 128] float32.

Per-core math (batch b):
  qT = (x_b @ Wq)^T, kT = (x_b @ Wk)^T, vT = (x_b @ Wv)^T   (Dh on partitions)
  per 128-token block j:
    v_j   = transpose(vT_j)                    (PE transpose, token-major)
    sT_j  = scoresT[k,q] = sum_d kT[d,k] qT[d,q]
    PT_j  = exp(sT_j / sqrt(Dh))               (no max-subtraction; logits are
                                                O(10) here, softmax algebra is
                                                exact without it)
    o'_j  = PT_j^T @ [v_j | 1 | 1]             (ones columns give row sums;
                                                two of them because f32r
                                                matmuls need an even free dim)
    out_j = o'_j[:, :128] * (1 / o'_j[:, 128])

Dtype strategy (MM_MODE):
  "bf16": projections in bf16 (1 cyc/row on the PE), attention chain in
          float32r (single-pass fp32_mode=HIGH, ~13-14 mantissa bits).
  "f32r": everything float32r (2 cyc/row projections).
  "f32":  everything fp32 (2-pass matmuls, slowest, exact).

The host pre-transposes x[b] to xT [D, S] so every device DMA is contiguous,
and shards batch b -> core b. The identity matrix for PE transposes comes
from the host too (affine_select cannot legally produce f32r data).
"""

import numpy as np
from contextlib import ExitStack

B, S, D, Dh, BLOCK = 8, 4096, 1024, 128, 128
KC = D // 128  # contraction chunks of 128
MT = 512       # token tile (moving free dim of projection matmuls)
NMT = S // MT
JT = MT // BLOCK
SCALE = float(1.0 / np.sqrt(np.float32(Dh)))

MM_MODE = "bf16"

_CACHE = {}


def _build(mode):
    import concourse.bass as bass
    import concourse.mybir as mybir
    import concourse.tile as tile
    from concourse import bacc

    f32 = mybir.dt.float32
    f32r = mybir.dt.float32r
    bf16 = mybir.dt.bfloat16
    ts = bass.ts

    # dtype of the x/W data path feeding the projection matmuls
    proj_dt = {"f32": f32, "f32r": f32r, "bf16": bf16}[mode]
    # dtype of the attention chain (qT/kT/vT staging, v, P, identity)
    attn_dt = f32 if mode == "f32" else f32r

    nc = bacc.Bacc("TRN2", target_bir_lowering=False, debug=False)

    xT = nc.dram_tensor("xT", [D, S], proj_dt, kind="ExternalInput").ap()
    ident_d = nc.dram_tensor("ident", [128, 128], attn_dt, kind="ExternalInput").ap()
    wq = nc.dram_tensor("wq", [D, Dh], proj_dt, kind="ExternalInput").ap()
    wk = nc.dram_tensor("wk", [D, Dh], proj_dt, kind="ExternalInput").ap()
    wv = nc.dram_tensor("wv", [D, Dh], proj_dt, kind="ExternalInput").ap()
    out = nc.dram_tensor("out", [S, Dh], f32, kind="ExternalOutput").ap()

    with tile.TileContext(nc) as tc, ExitStack() as ctx:
        wpool = ctx.enter_context(tc.tile_pool(name="w", bufs=1))
        cpool = ctx.enter_context(tc.tile_pool(name="const", bufs=1))
        xpool = ctx.enter_context(tc.tile_pool(name="x", bufs=2))
        spool = ctx.enter_context(tc.tile_pool(name="s", bufs=2))
        apool = ctx.enter_context(tc.tile_pool(name="a", bufs=3))
        opool = ctx.enter_context(tc.tile_pool(name="o", bufs=2))
        ppool = ctx.enter_context(tc.tile_pool(name="proj_ps", bufs=3, space="PSUM"))
        qpool = ctx.enter_context(tc.tile_pool(name="attn_ps", bufs=5, space="PSUM"))

        ident = cpool.tile([128, 128], attn_dt, tag="ident")
        nc.sync.dma_start(ident[:], ident_d[:])

        # Weights, chunked along the contraction dim: w_t[:, k, :] is the
        # [128, Dh] stationary tile for contraction chunk k. Split the DMAs
        # per chunk so the first projection matmul only waits for chunk 0.
        w_ts = []
        for name, w in (("wq", wq), ("wk", wk), ("wv", wv)):
            w_t = wpool.tile([128, KC, Dh], proj_dt, tag=f"w_{name}")
            wv_view = w.rearrange("(k p) d -> p k d", p=128)
            for k in range(KC):
                nc.sync.dma_start(w_t[:, k, :], wv_view[:, k, :])
            w_ts.append(w_t)
        wq_t, wk_t, wv_t = w_ts

        for mt in range(NMT):
            m0 = mt * MT
            # x^T slice for this token tile, one DMA per contraction chunk
            xt = xpool.tile([128, KC, MT], proj_dt, tag="xt")
            xTv = xT.rearrange("(k p) s -> p k s", p=128)
            for k in range(KC):
                nc.sync.dma_start(xt[:, k, :], xTv[:, k, m0 : m0 + MT])

            # Projections (Dh on partitions): pT[d, m] = sum_k W[k, d] xT[k, m]
            pT_sbs = []
            for w_t, tag, copy_eng in (
                (wq_t, "qT", nc.vector),
                (wk_t, "kT", nc.scalar),
                (wv_t, "vT", nc.vector),
            ):
                pT_ps = ppool.tile([128, MT], f32, tag="proj")
                for k in range(KC):
                    nc.tensor.matmul(
                        pT_ps[:],
                        w_t[:, k, :],
                        xt[:, k, :],
                        start=(k == 0),
                        stop=(k == KC - 1),
                    )
                pT_sb = spool.tile([128, MT], attn_dt, tag=tag)
                if copy_eng is nc.scalar:
                    nc.scalar.copy(pT_sb[:], pT_ps[:])
                else:
                    nc.vector.tensor_copy(pT_sb[:], pT_ps[:])
                pT_sbs.append(pT_sb)
            qT_sb, kT_sb, vT_sb = pT_sbs

            # Attention on the JT blocks of this token tile. Output blocks
            # collect into one [128, JT*BLOCK] tile -> single DMA per m-tile.
            o_mt = opool.tile([128, JT, BLOCK], f32, tag="o_mt")
            for j in range(JT):
                blk = ts(j, BLOCK)
                # token-major v block via PE transpose
                v_ps = qpool.tile([128, BLOCK], attn_dt, tag="attn")
                nc.tensor.transpose(v_ps[:], vT_sb[:, blk], ident[:])
                # f32r matmuls need an even moving free dim: two ones columns
                v_sb = apool.tile([128, BLOCK + 2], attn_dt, tag="v")
                nc.vector.tensor_copy(v_sb[:, 0:BLOCK], v_ps[:])
                ones_col = v_sb[:, BLOCK : BLOCK + 2]
                nc.vector.memset(
                    ones_col.bitcast(f32) if attn_dt == f32r else ones_col, 1.0
                )

                # scoresT[k, q] = sum_d kT[d, k] qT[d, q]
                sT_ps = qpool.tile([128, BLOCK], f32, tag="attn")
                nc.tensor.matmul(
                    sT_ps[:], kT_sb[:, blk], qT_sb[:, blk], start=True, stop=True
                )
                PT = apool.tile([128, BLOCK], attn_dt, tag="PT")
                nc.scalar.activation(
                    PT[:], sT_ps[:], mybir.ActivationFunctionType.Exp, scale=SCALE
                )

                # o'[q, :Dh] = unnormalized attn output; o'[q, Dh] = row sum
                o_ps = qpool.tile([128, BLOCK + 2], f32, tag="attn")
                nc.tensor.matmul(o_ps[:], PT[:], v_sb[:], start=True, stop=True)

                r_sb = apool.tile([128, 1], f32, tag="r")
                nc.vector.reciprocal(r_sb[:], o_ps[:, BLOCK : BLOCK + 1])
                nc.vector.tensor_scalar_mul(o_mt[:, j, :], o_ps[:, 0:BLOCK], r_sb[:])

            # out[m0 + c*BLOCK + p, d] <- o_mt[p, c, d]
            out_view = out[m0 : m0 + MT, :].rearrange("(c p) d -> p c d", p=BLOCK)
            nc.scalar.dma_start(out_view, o_mt[:])

    nc.compile()
    return nc


def _get_nc():
    if MM_MODE not in _CACHE:
        _CACHE[MM_MODE] = _build(MM_MODE)
    return _CACHE[MM_MODE]


def _casts():
    if MM_MODE == "bf16":
        import ml_dtypes

        proj_cast = lambda a: np.asarray(a, dtype=ml_dtypes.bfloat16)
    else:
        proj_cast = lambda a: np.ascontiguousarray(np.asarray(a, dtype=np.float32))
    attn_cast = lambda a: np.ascontiguousarray(np.asarray(a, dtype=np.float32))
    return proj_cast, attn_cast


def make_in_maps(x, Wq, Wk, Wv):
    proj_cast, attn_cast = _casts()
    wq_h, wk_h, wv_h = proj_cast(Wq), proj_cast(Wk), proj_cast(Wv)
    ident_h = attn_cast(np.eye(128, dtype=np.float32))
    return [
        {
            "xT": np.ascontiguousarray(proj_cast(np.asarray(x[b]).T)),
            "wq": wq_h,
            "wk": wk_h,
            "wv": wv_h,
            "ident": ident_h,
        }
        for b in range(B)
    ]


def kernel(x, Wq, Wk, Wv):
    from concourse.bass_utils import run_bass_kernel_spmd

    nc = _get_nc()
    in_maps = make_in_maps(x, Wq, Wk, Wv)
    res = run_bass_kernel_spmd(nc, in_maps, list(range(B))).results
    return np.stack([res[b]["out"] for b in range(B)], axis=0)


# revision 10
# speedup vs baseline: 1.2472x; 1.0993x over previous
"""Block-sparse (block-diagonal local) attention head for Trainium2, 8-way
data-parallel over the batch dimension (one batch element per NeuronCore).

Contract: kernel(**inputs) takes the FULL inputs from setup_inputs() and
returns the FULL output of reference(): out [8, 4096, 128] float32.

Per-core math (batch b):
  qT = (x_b @ Wq)^T, kT = (x_b @ Wk)^T, vT = (x_b @ Wv)^T   (Dh on partitions)
  per 128-token block j:
    v_j   = transpose(vT_j)                    (PE transpose, token-major)
    sT_j  = scoresT[k,q] = sum_d kT[d,k] qT[d,q]
    PT_j  = exp(sT_j / sqrt(Dh))               (no max-subtraction; logits are
                                                O(10) here, softmax algebra is
                                                exact without it)
    o'_j  = PT_j^T @ [v_j | 1 | 1]             (ones columns give row sums;
                                                two of them because f32r
                                                matmuls need an even free dim)
    out_j = o'_j[:, :128] * (1 / o'_j[:, 128])

Dtype strategy (MM_MODE):
  "bf16": projections in bf16 (1 cyc/row on the PE), attention chain in
          float32r (single-pass fp32_mode=HIGH, ~13-14 mantissa bits).
  "f32r": everything float32r (2 cyc/row projections).
  "f32":  everything fp32 (2-pass matmuls, slowest, exact).

Host-side prep (sharding freedom): batch b -> core b; x[b] is transposed and
repacked to xp [128, KC, S] (partition-major, so each DMA reads long
contiguous runs); the three projection weights are repacked into one
wp [128, 3, KC, Dh] tensor; the PE-transpose identity ships from the host
(affine_select cannot legally produce f32r data).
"""

import numpy as np
from contextlib import ExitStack

B, S, D, Dh, BLOCK = 8, 4096, 1024, 128, 128
KC = D // 128  # contraction chunks of 128
MT = 512       # token tile (moving free dim of projection matmuls)
STS = 1024     # token super-tile per x DMA
NST = S // STS
JT = MT // BLOCK
SCALE = float(1.0 / np.sqrt(np.float32(Dh)))

MM_MODE = "bf16"

_CACHE = {}


def _build(mode):
    import concourse.bass as bass
    import concourse.mybir as mybir
    import concourse.tile as tile
    from concourse import bacc

    f32 = mybir.dt.float32
    f32r = mybir.dt.float32r
    bf16 = mybir.dt.bfloat16
    ts = bass.ts

    # dtype of the x/W data path feeding the projection matmuls
    proj_dt = {"f32": f32, "f32r": f32r, "bf16": bf16}[mode]
    # dtype of the attention chain (qT/kT/vT staging, v, P, identity)
    attn_dt = f32 if mode == "f32" else f32r

    nc = bacc.Bacc("TRN2", target_bir_lowering=False, debug=False)

    xp = nc.dram_tensor("xp", [128, KC, S], proj_dt, kind="ExternalInput").ap()
    ident_d = nc.dram_tensor("ident", [128, 128], attn_dt, kind="ExternalInput").ap()
    wp = nc.dram_tensor("wp", [128, 3, KC, Dh], proj_dt, kind="ExternalInput").ap()
    out = nc.dram_tensor("out", [S, Dh], f32, kind="ExternalOutput").ap()

    with tile.TileContext(nc) as tc, ExitStack() as ctx:
        wpool = ctx.enter_context(tc.tile_pool(name="w", bufs=1))
        cpool = ctx.enter_context(tc.tile_pool(name="const", bufs=1))
        xpool = ctx.enter_context(tc.tile_pool(name="x", bufs=2))
        spool = ctx.enter_context(tc.tile_pool(name="s", bufs=2))
        apool = ctx.enter_context(tc.tile_pool(name="a", bufs=3))
        opool = ctx.enter_context(tc.tile_pool(name="o", bufs=2))
        ppool = ctx.enter_context(tc.tile_pool(name="proj_ps", bufs=3, space="PSUM"))
        qpool = ctx.enter_context(tc.tile_pool(name="attn_ps", bufs=5, space="PSUM"))

        # Weights (one DMA, contiguous per partition) + identity on the
        # scalar (ACT) HWDGE ring; x streams on the sync (SP) ring.
        wp_t = wpool.tile([128, 3, KC, Dh], proj_dt, tag="wp")
        nc.scalar.dma_start(wp_t[:, 0:1], wp[:, 0:1])  # wq first: gates first MM
        nc.scalar.dma_start(wp_t[:, 1:3], wp[:, 1:3])
        ident = cpool.tile([128, 128], attn_dt, tag="ident")
        nc.scalar.dma_start(ident[:], ident_d[:])

        for st in range(NST):
            s0 = st * STS
            # x super-tile [128, KC, STS]; first one split in two so the
            # first projection matmuls start after half a super-tile
            xt = xpool.tile([128, KC, STS], proj_dt, tag="xt")
            if st == 0:
                nc.sync.dma_start(xt[:, 0:1], xp[:, 0:1, s0 : s0 + STS])
                nc.sync.dma_start(xt[:, 1:KC], xp[:, 1:KC, s0 : s0 + STS])
            else:
                nc.sync.dma_start(xt[:], xp[:, :, s0 : s0 + STS])

            for sub in range(STS // MT):
                moff = sub * MT
                m0 = s0 + moff

                # Projections (Dh on partitions):
                # pT[d, m] = sum_k W[k, d] xT[k, m]
                pT_sbs = []
                for wi, tag, copy_eng in (
                    (0, "qT", nc.vector),
                    (1, "kT", nc.scalar),
                    (2, "vT", nc.vector),
                ):
                    pT_ps = ppool.tile([128, MT], f32, tag="proj")
                    for k in range(KC):
                        nc.tensor.matmul(
                            pT_ps[:],
                            wp_t[:, wi, k, :],
                            xt[:, k, moff : moff + MT],
                            start=(k == 0),
                            stop=(k == KC - 1),
                        )
                    pT_sb = spool.tile([128, MT], attn_dt, tag=tag)
                    if copy_eng is nc.scalar:
                        nc.scalar.copy(pT_sb[:], pT_ps[:])
                    else:
                        nc.vector.tensor_copy(pT_sb[:], pT_ps[:])
                    pT_sbs.append(pT_sb)
                qT_sb, kT_sb, vT_sb = pT_sbs

                # Attention on the JT blocks of this m-tile. Output blocks
                # collect into one [128, JT*BLOCK] tile -> one DMA per m-tile.
                o_mt = opool.tile([128, JT, BLOCK], f32, tag="o_mt")
                for j in range(JT):
                    blk = ts(j, BLOCK)
                    # token-major v block via PE transpose
                    v_ps = qpool.tile([128, BLOCK], attn_dt, tag="attn")
                    nc.tensor.transpose(v_ps[:], vT_sb[:, blk], ident[:])
                    # f32r matmuls need an even moving free dim:
                    # two ones columns
                    v_sb = apool.tile([128, BLOCK + 2], attn_dt, tag="v")
                    nc.vector.tensor_copy(v_sb[:, 0:BLOCK], v_ps[:])
                    ones_col = v_sb[:, BLOCK : BLOCK + 2]
                    nc.vector.memset(
                        ones_col.bitcast(f32) if attn_dt == f32r else ones_col,
                        1.0,
                    )

                    # scoresT[k, q] = sum_d kT[d, k] qT[d, q]
                    sT_ps = qpool.tile([128, BLOCK], f32, tag="attn")
                    nc.tensor.matmul(
                        sT_ps[:], kT_sb[:, blk], qT_sb[:, blk], start=True, stop=True
                    )
                    PT = apool.tile([128, BLOCK], attn_dt, tag="PT")
                    nc.scalar.activation(
                        PT[:], sT_ps[:], mybir.ActivationFunctionType.Exp, scale=SCALE
                    )

                    # o'[q, :Dh] = unnormalized attn out; o'[q, Dh] = row sum
                    o_ps = qpool.tile([128, BLOCK + 2], f32, tag="attn")
                    nc.tensor.matmul(o_ps[:], PT[:], v_sb[:], start=True, stop=True)

                    r_sb = apool.tile([128, 1], f32, tag="r")
                    nc.vector.reciprocal(r_sb[:], o_ps[:, BLOCK : BLOCK + 1])
                    nc.vector.tensor_scalar_mul(
                        o_mt[:, j, :], o_ps[:, 0:BLOCK], r_sb[:]
                    )

                # out[m0 + c*BLOCK + p, d] <- o_mt[p, c, d]
                out_view = out[m0 : m0 + MT, :].rearrange(
                    "(c p) d -> p c d", p=BLOCK
                )
                nc.scalar.dma_start(out_view, o_mt[:])

    nc.compile()
    return nc


def _get_nc():
    if MM_MODE not in _CACHE:
        _CACHE[MM_MODE] = _build(MM_MODE)
    return _CACHE[MM_MODE]


def _casts():
    if MM_MODE == "bf16":
        import ml_dtypes

        proj_np = ml_dtypes.bfloat16
    else:
        proj_np = np.float32
    return proj_np


def make_in_maps(x, Wq, Wk, Wv):
    proj_np = _casts()
    # wp[p, i, k, d] = W_i[k*128 + p, d]
    wp = np.stack(
        [np.asarray(w).reshape(KC, 128, Dh).transpose(1, 0, 2) for w in (Wq, Wk, Wv)],
        axis=1,
    )
    wp_h = np.ascontiguousarray(wp.astype(proj_np))
    ident_h = np.eye(128, dtype=np.float32)
    x = np.asarray(x)
    maps = []
    for b in range(B):
        # xp[p, k, s] = x[b].T[k*128 + p, s]
        xp = np.asarray(x[b], dtype=proj_np).T.reshape(KC, 128, S).transpose(1, 0, 2)
        maps.append(
            {
                "xp": np.ascontiguousarray(xp),
                "wp": wp_h,
                "ident": ident_h,
            }
        )
    return maps


def kernel(x, Wq, Wk, Wv):
    from concourse.bass_utils import run_bass_kernel_spmd

    nc = _get_nc()
    in_maps = make_in_maps(x, Wq, Wk, Wv)
    res = run_bass_kernel_spmd(nc, in_maps, list(range(B))).results
    return np.stack([res[b]["out"] for b in range(B)], axis=0)


# revision 11
# speedup vs baseline: 1.3474x; 1.0804x over previous
"""Block-sparse (block-diagonal local) attention head for Trainium2, 8-way
data-parallel over the batch dimension (one batch element per NeuronCore).

Contract: kernel(**inputs) takes the FULL inputs from setup_inputs() and
returns the FULL output of reference(): out [8, 4096, 128] float32.

Per-core math (batch b):
  qT = (x_b @ Wq)^T, kT = (x_b @ Wk)^T, vT = (x_b @ Wv)^T   (Dh on partitions)
  per 128-token block j:
    v_j   = transpose(vT_j)                    (PE transpose, token-major)
    sT_j  = scoresT[k,q] = sum_d kT[d,k] qT[d,q]
    PT_j  = exp(sT_j / sqrt(Dh))               (no max-subtraction; logits are
                                                O(10) here, softmax algebra is
                                                exact without it)
    o'_j  = PT_j^T @ [v_j | 1 | 1]             (ones columns give row sums;
                                                two of them because f32r
                                                matmuls need an even free dim)
    out_j = o'_j[:, :128] * (1 / o'_j[:, 128])

Dtype strategy (MM_MODE):
  "bf16": projections in bf16 (1 cyc/row on the PE), attention chain in
          float32r (single-pass fp32_mode=HIGH, ~13-14 mantissa bits).
  "f32r": everything float32r (2 cyc/row projections).
  "f32":  everything fp32 (2-pass matmuls, slowest, exact).

Host-side prep (sharding freedom): batch b -> core b; x[b] is transposed and
repacked to xp [128, KC, S] (partition-major, so each DMA reads long
contiguous runs); the three projection weights are repacked into one
wp [128, 3, KC, Dh] tensor; the PE-transpose identity ships from the host
(affine_select cannot legally produce f32r data).
"""

import numpy as np
from contextlib import ExitStack

B, S, D, Dh, BLOCK = 8, 4096, 1024, 128, 128
KC = D // 128  # contraction chunks of 128
MT = 512       # token tile (moving free dim of projection matmuls)
STS = 1024     # token super-tile per x DMA
NST = S // STS
JT = MT // BLOCK
SCALE = float(1.0 / np.sqrt(np.float32(Dh)))

MM_MODE = "bf16"

_CACHE = {}


def _build(mode):
    import concourse.bass as bass
    import concourse.mybir as mybir
    import concourse.tile as tile
    from concourse import bacc

    f32 = mybir.dt.float32
    f32r = mybir.dt.float32r
    bf16 = mybir.dt.bfloat16
    ts = bass.ts

    # dtype of the x/W data path feeding the projection matmuls
    proj_dt = {"f32": f32, "f32r": f32r, "bf16": bf16}[mode]
    # dtype of the scores chain (qT/kT staging)
    attn_dt = f32 if mode == "f32" else f32r
    # dtype of the v/transpose/o' chain (P, v, identity): bf16 rounding of
    # P (in [0,1]) and v adds nothing over the bf16 projection error
    trans_dt = bf16 if mode == "bf16" else attn_dt

    nc = bacc.Bacc("TRN2", target_bir_lowering=False, debug=False)

    xp = nc.dram_tensor("xp", [128, KC, S], proj_dt, kind="ExternalInput").ap()
    ident_d = nc.dram_tensor("ident", [128, 128], trans_dt, kind="ExternalInput").ap()
    wp = nc.dram_tensor("wp", [128, 3, KC, Dh], proj_dt, kind="ExternalInput").ap()
    out = nc.dram_tensor("out", [S, Dh], f32, kind="ExternalOutput").ap()

    with tile.TileContext(nc) as tc, ExitStack() as ctx:
        wpool = ctx.enter_context(tc.tile_pool(name="w", bufs=1))
        cpool = ctx.enter_context(tc.tile_pool(name="const", bufs=1))
        xpool = ctx.enter_context(tc.tile_pool(name="x", bufs=2))
        spool = ctx.enter_context(tc.tile_pool(name="s", bufs=2))
        apool = ctx.enter_context(tc.tile_pool(name="a", bufs=3))
        opool = ctx.enter_context(tc.tile_pool(name="o", bufs=2))
        ppool = ctx.enter_context(tc.tile_pool(name="proj_ps", bufs=3, space="PSUM"))
        qpool = ctx.enter_context(tc.tile_pool(name="attn_ps", bufs=5, space="PSUM"))

        # Weights (one DMA, contiguous per partition) + identity on the
        # scalar (ACT) HWDGE ring; x streams on the sync (SP) ring.
        wp_t = wpool.tile([128, 3, KC, Dh], proj_dt, tag="wp")
        nc.scalar.dma_start(wp_t[:, 0:1], wp[:, 0:1])  # wq first: gates first MM
        nc.scalar.dma_start(wp_t[:, 1:3], wp[:, 1:3])
        ident = cpool.tile([128, 128], trans_dt, tag="ident")
        nc.scalar.dma_start(ident[:], ident_d[:])

        for st in range(NST):
            s0 = st * STS
            # x super-tile [128, KC, STS]; first one split in two so the
            # first projection matmuls start after half a super-tile
            xt = xpool.tile([128, KC, STS], proj_dt, tag="xt")
            if st == 0:
                # split across chunks and both HWDGE rings: the first
                # matmul gates only on chunk 0, and the two rings halve
                # the latency of the rest
                nc.sync.dma_start(xt[:, 0:1], xp[:, 0:1, s0 : s0 + STS])
                nc.sync.dma_start(xt[:, 1:4], xp[:, 1:4, s0 : s0 + STS])
                nc.scalar.dma_start(xt[:, 4:KC], xp[:, 4:KC, s0 : s0 + STS])
            else:
                nc.sync.dma_start(xt[:], xp[:, :, s0 : s0 + STS])

            for sub in range(STS // MT):
                moff = sub * MT
                m0 = s0 + moff

                # Projections (Dh on partitions):
                # pT[d, m] = sum_k W[k, d] xT[k, m]
                pT_sbs = []
                for wi, tag, copy_eng, sb_dt in (
                    (0, "qT", nc.vector, attn_dt),
                    (1, "kT", nc.scalar, attn_dt),
                    (2, "vT", nc.vector, trans_dt),
                ):
                    pT_ps = ppool.tile([128, MT], f32, tag="proj")
                    for k in range(KC):
                        nc.tensor.matmul(
                            pT_ps[:],
                            wp_t[:, wi, k, :],
                            xt[:, k, moff : moff + MT],
                            start=(k == 0),
                            stop=(k == KC - 1),
                        )
                    pT_sb = spool.tile([128, MT], sb_dt, tag=tag)
                    if copy_eng is nc.scalar:
                        nc.scalar.copy(pT_sb[:], pT_ps[:])
                    else:
                        nc.vector.tensor_copy(pT_sb[:], pT_ps[:])
                    pT_sbs.append(pT_sb)
                qT_sb, kT_sb, vT_sb = pT_sbs

                # Attention on the JT blocks of this m-tile. Output blocks
                # collect into one [128, JT*BLOCK] tile -> one DMA per m-tile.
                o_mt = opool.tile([128, JT, BLOCK], f32, tag="o_mt")
                for j in range(JT):
                    blk = ts(j, BLOCK)
                    # token-major v block via PE transpose
                    v_ps = qpool.tile([128, BLOCK], trans_dt, tag="attn")
                    nc.tensor.transpose(v_ps[:], vT_sb[:, blk], ident[:])
                    # f32r matmuls need an even moving free dim:
                    # two ones columns
                    v_sb = apool.tile([128, BLOCK + 2], trans_dt, tag="v")
                    nc.vector.tensor_copy(v_sb[:, 0:BLOCK], v_ps[:])
                    ones_col = v_sb[:, BLOCK : BLOCK + 2]
                    nc.vector.memset(
                        ones_col.bitcast(f32) if trans_dt == f32r else ones_col,
                        1.0,
                    )

                    # scoresT[k, q] = sum_d kT[d, k] qT[d, q]
                    sT_ps = qpool.tile([128, BLOCK], f32, tag="attn")
                    nc.tensor.matmul(
                        sT_ps[:], kT_sb[:, blk], qT_sb[:, blk], start=True, stop=True
                    )
                    PT = apool.tile([128, BLOCK], trans_dt, tag="PT")
                    nc.scalar.activation(
                        PT[:], sT_ps[:], mybir.ActivationFunctionType.Exp, scale=SCALE
                    )

                    # o'[q, :Dh] = unnormalized attn out; o'[q, Dh] = row sum
                    o_ps = qpool.tile([128, BLOCK + 2], f32, tag="attn")
                    nc.tensor.matmul(o_ps[:], PT[:], v_sb[:], start=True, stop=True)

                    r_sb = apool.tile([128, 1], f32, tag="r")
                    nc.vector.reciprocal(r_sb[:], o_ps[:, BLOCK : BLOCK + 1])
                    nc.vector.tensor_scalar_mul(
                        o_mt[:, j, :], o_ps[:, 0:BLOCK], r_sb[:]
                    )

                # out[m0 + c*BLOCK + p, d] <- o_mt[p, c, d]
                out_view = out[m0 : m0 + MT, :].rearrange(
                    "(c p) d -> p c d", p=BLOCK
                )
                nc.scalar.dma_start(out_view, o_mt[:])

    nc.compile()
    return nc


def _get_nc():
    if MM_MODE not in _CACHE:
        _CACHE[MM_MODE] = _build(MM_MODE)
    return _CACHE[MM_MODE]


def _casts():
    if MM_MODE == "bf16":
        import ml_dtypes

        proj_np = ml_dtypes.bfloat16
    else:
        proj_np = np.float32
    return proj_np


def make_in_maps(x, Wq, Wk, Wv):
    proj_np = _casts()
    # wp[p, i, k, d] = W_i[k*128 + p, d]
    wp = np.stack(
        [np.asarray(w).reshape(KC, 128, Dh).transpose(1, 0, 2) for w in (Wq, Wk, Wv)],
        axis=1,
    )
    wp_h = np.ascontiguousarray(wp.astype(proj_np))
    ident_h = np.eye(128, dtype=proj_np if MM_MODE == "bf16" else np.float32)
    x = np.asarray(x)
    maps = []
    for b in range(B):
        # xp[p, k, s] = x[b].T[k*128 + p, s]
        xp = np.asarray(x[b], dtype=proj_np).T.reshape(KC, 128, S).transpose(1, 0, 2)
        maps.append(
            {
                "xp": np.ascontiguousarray(xp),
                "wp": wp_h,
                "ident": ident_h,
            }
        )
    return maps


def kernel(x, Wq, Wk, Wv):
    from concourse.bass_utils import run_bass_kernel_spmd

    nc = _get_nc()
    in_maps = make_in_maps(x, Wq, Wk, Wv)
    res = run_bass_kernel_spmd(nc, in_maps, list(range(B))).results
    return np.stack([res[b]["out"] for b in range(B)], axis=0)
